# revision 69
# baseline (speedup 1.0000x reference)
"""Trainium2 Bass kernel for nn_BasicTransformerBlock (key-frame cross attention).

Reference computation (B=16 frames, S=1024, C=320, H=8 heads, D=40):
    q = x @ Wq.T ; k = x @ Wk.T ; v = x @ Wv.T
    k, v are taken from frame `kf` only and shared by every frame
    out = softmax(q k^T / sqrt(D)) v     (per frame, per head)
    y = out @ Wo.T + bo

Sharding: data-parallel over frames - 8 cores x 2 frames each. Every core
redundantly computes K/V from the key frame (cheap) so no collectives are
needed; outputs just concatenate.

Per-core design (v2, exp-wall balanced):
  - ScalarE (Act) does almost nothing but the 16.8M exps (2f x 8h x 1024^2
    per core) in [128,1024] psum->sbuf passes - the ~110us roofline. 24 of
    the 128 exp chunks are offloaded to the DVE as a Schraudolph fast-exp
    (one tensor_scalar writing int16 bits that ARE the bf16 exp, ~3%
    pointwise, ~1e-2 end to end after softmax renormalization); their
    score tiles go through the shared psum pool so the 2-deep scores-pool
    WAR rotation - the per-step pacing floor - skips those steps entirely
    and the Act stream compresses over the holes.
  - Scores run fp32r [t,s]-transposed: one 64-row head quadrant of kT
    against qT, N=512 halves (full PE rate).
  - PV runs TRANSPOSED: out[s,d] = sum_t pt[t,s] v[t,d] with pt (bf16) as
    the stationary operand and v (bf16, 40 real dims + a ones column) as
    the moving operand. Matmul cost is (output free size) x (chunks), so
    N=41 instead of a 512-wide padded layout cuts PV cycles 3x, and the
    ones column makes psum col 40 the softmax denominator. PSUM start=True
    zeroes a whole 2KB bank, so each pvz bank carries exactly ONE
    accumulation group (first matmul starts it, last one stops it).
  - Normalization is a [128,8] reciprocal plus one broadcast multiply per
    (frame,head) on DVE - per-partition scalars, no partition broadcasts.
  - o[s,d] is transposed back to oT[d,s] on the PE (is_transpose against a
    host-supplied identity, one bank-group per psum tile), then the
    O-projection contracts the padded 512 rows in fp32r; bias-add happens
    on DVE for frame 0 and on the then-idle Act at the tail.
  - PSUM: 2x[128,1024] score tiles + 2x[128,328] pv+Z tiles + 2x[128,512]
    shared (projections / transposes / O-proj / offloaded scores) = 8
    banks exactly.
  - Startup: DMA order follows the q-side/k-side critical path with the
    projections chasing individual chunk transfers, fp32 warm-up matmuls
    hold the PE p-state at full clock, and a dummy exp hoists the 1.3us
    activation-table load to t~0. Tail: three frame-1 O-proj groups
    pre-accumulate their first 3 head-pair chunks during the final unit.
  - GPSIMD cannot touch PSUM and only flat F32 memsets compile, so all
    psum drains are DVE/Act and constant fills go through broadcast
    copies.
"""

import os
import sys

import numpy as np

try:
    import concourse  # noqa: F401
except ImportError:  # pragma: no cover
    for _p in ("/opt/trn_rl_repo", os.path.dirname(os.path.abspath(__file__))):
        if os.path.isdir(os.path.join(_p, "concourse")):
            sys.path.insert(0, _p)
            break

import concourse.mybir as mybir  # noqa: E402
import concourse.tile as tile  # noqa: E402
from concourse import bacc  # noqa: E402
from concourse import bass_utils  # noqa: E402

F32 = mybir.dt.float32
F32R = mybir.dt.float32r
BF16 = mybir.dt.bfloat16

S = 1024          # sequence length per frame
C = 320           # channels
H = 8             # heads
D = 40            # head dim
DP = 64           # padded head dim (q/k score layout + O-proj rows)
CP = H * DP       # 512
VW = D + 1        # v block width: 40 dims + ones column (denominator)
NCORES = 8
FPC = 2           # frames per core
SCALE = float(D) ** -0.5

CI = [(0, 128), (128, 128), (256, 64)]    # c_in chunks of 320
CO = [(0, 128), (128, 128), (256, 64)]    # c_out chunks of 320

# Schraudolph fast-exp constants (bf16 bit space), scale folded in:
# bf16bits(exp(s*SCALE)) ~ int16(s*SCALE*128/ln2 + 127*128 - C0 + 0.5)
import math as _math
_SCH_C0 = 64.0 * (1.0 - (_math.log(_math.log(2.0)) + 1.0) / _math.log(2.0))
SCH_A = SCALE * 128.0 / _math.log(2.0)
SCH_B = 127.0 * 128.0 - _SCH_C0 + 0.5
# (frame*H+head) -> set of t-tiles whose exp runs on the DVE instead of Act.
# ~1.5 per unit, avoiding tt=7 (it gates the unit epilogue).
# tt 3..5 land where the DVE queue is quiet (unit-epilogue copies occupy
# it during tt 0..2), so the fast-exp finishes before its PV needs it
# ~1.5 chunks per unit; more offload stops paying (the PE becomes the
# wall) while the approximation error keeps growing, and odd units keep a
# single offload so the psh slots shared with transposes / tail O-proj
# pre-runs never form an allocation cycle (build-time deadlock otherwise).
OFFLOAD_TT = {
    u: ({3, 5} if u % 2 == 0 else {4})
    for u in range(16)
}

_NC_CACHE: dict = {}
LAST_RESULTS = None


def _build(loop_n: int = 1):
    nc = bacc.Bacc("TRN2", target_bir_lowering=False, debug=False)

    CPAD = 384
    xt0 = nc.dram_tensor("xt0", [CPAD, S], F32R, kind="ExternalInput")
    xtf = nc.dram_tensor("xtf", [FPC, CPAD, S], F32R, kind="ExternalInput")
    wkq = nc.dram_tensor("wkq", [CPAD, 2 * CP], F32R, kind="ExternalInput")
    wvd = nc.dram_tensor("wvd", [CPAD, C], F32R, kind="ExternalInput")
    wo = nc.dram_tensor("wo", [CP, C], F32R, kind="ExternalInput")
    bo = nc.dram_tensor("bo", [CPAD], F32, kind="ExternalInput")
    ident = nc.dram_tensor("ident", [128, 128], F32R, kind="ExternalInput")
    yt = nc.dram_tensor("yt", [FPC, C, S], F32, kind="ExternalOutput")

    with tile.TileContext(nc) as tc:
        with (
            tc.tile_pool(name="pconst", bufs=1) as pconst,
            tc.tile_pool(name="pqk", bufs=1) as pqk,
            tc.tile_pool(name="pvs", bufs=1) as pvs,
            tc.tile_pool(name="ppt", bufs=2) as ppt,
            tc.tile_pool(name="po", bufs=2) as po,
            tc.tile_pool(name="pot", bufs=1) as pot,
            tc.tile_pool(name="prc", bufs=4) as prc,
            tc.tile_pool(name="py", bufs=6) as py,
            tc.tile_pool(name="psc", bufs=2, space="PSUM") as psc,
            tc.tile_pool(name="ppv", bufs=2, space="PSUM") as ppv,
            tc.tile_pool(name="psh", bufs=2, space="PSUM") as psh,
        ):
          for it in range(loop_n):
            P = f"{it}_"

            # ---- constants / inputs staged in SBUF --------------------
            ident_sb = pconst.tile([128, 128], F32R, name=f"{P}ident", tag="ident")
            bo_all = pconst.tile([128, 3], F32, name=f"{P}bo", tag="bo")
            bo_sb = [bo_all[0:cn, m:m + 1] for m, (cs, cn) in enumerate(CO)]

            def dma_cols(tile_t, dram_ap, width, c0, c1):
                """Columns [c0:c1] of a [384, width] dram tensor into the
                matching slice of a [128, 3*width] folded tile (one 3-level
                DMA; splitting off the zero-pad rows costs more in per-DMA
                overhead than the bytes save)."""
                nc.sync.dma_start(
                    tile_t[:].rearrange("p (c w) -> p c w", w=width)[:, :, c0:c1],
                    dram_ap.rearrange("(c p) w -> p c w", p=128)[:, :, c0:c1],
                )

            def mk_tile3(pool, nm, width, dt=F32R):
                t = pool.tile([128, 3 * width], dt, name=f"{P}{nm}", tag=nm)
                return t, [t[0:cn, ci * width:ci * width + width] for ci, (cs, cn) in enumerate(CI)]

            x0_t, x0_sb = mk_tile3(pconst, "x0a", S)
            wkq_t, wkq_v = mk_tile3(pconst, "wkqa", 2 * CP)
            wk_sb = [t[:, 0:CP] for t in wkq_v]
            wq_sb = [t[:, CP:2 * CP] for t in wkq_v]
            wv_t, wv_sb = mk_tile3(pconst, "wva", C)
            wo_all = pconst.tile([128, 4 * C], F32R, name=f"{P}wo", tag="wo")
            wo_sb = [wo_all[:, cp * C:(cp + 1) * C] for cp in range(4)]

            xf_parts = [
                pconst.tile([128, 3 * S], F32R, name=f"{P}xfa{f}", tag=f"xfa{f}")
                for f in range(FPC)
            ]
            xf_sb = [
                [xf_parts[f][0:cn, ci * S:ci * S + S] for ci, (cs, cn) in enumerate(CI)]
                for f in range(FPC)
            ]

            def load_xf_frame(f):
                nc.sync.dma_start(
                    xf_parts[f][:].rearrange("p (c w) -> p c w", w=S),
                    xtf.ap()[f].rearrange("(c p) w -> p c w", p=128),
                )

            # DMA issue order = the startup critical path. The DMA engines
            # are modeled as exclusive, so ident (tiny, feeds the warm-up
            # matmuls) and the small weight chunks go FIRST, then xf0 per
            # c-chunk (the q projection chases each chunk), then x0 in
            # 128-col t-chunks (the k projection chases those), with wv
            # slotted so the v projections stay ahead of the PV stream.
            dma_cols(wkq_t, wkq.ap(), 2 * CP, CP, CP + 128)      # wq m0
            xf0_tv = xf_parts[0][:].rearrange("p (c w) -> p c w", w=S)
            xf0_dv = xtf.ap()[0].rearrange("(c p) w -> p c w", p=128)
            for ci in range(2):                                   # xf0 c-chunks
                nc.sync.dma_start(xf0_tv[:, ci:ci + 1, :], xf0_dv[:, ci:ci + 1, :])
            nc.sync.dma_start(xf0_tv[0:64, 2:3, :], xf0_dv[0:64, 2:3, :])
            dma_cols(wkq_t, wkq.ap(), 2 * CP, 0, 128)            # wk m0
            for tc in range(2):                                   # x0 t 0:256
                dma_cols(x0_t, xt0.ap(), S, tc * 128, (tc + 1) * 128)
            dma_cols(wv_t, wvd.ap(), C, 0, C)                     # wv
            for tc in range(2, 8):                                # x0 t 256:1024
                dma_cols(x0_t, xt0.ap(), S, tc * 128, (tc + 1) * 128)
            nc.sync.dma_start(ident_sb[:], ident.ap())
            dma_cols(wkq_t, wkq.ap(), 2 * CP, 128, CP)            # wk rest
            dma_cols(wkq_t, wkq.ap(), 2 * CP, CP + 128, 2 * CP)   # wq rest
            nc.sync.dma_start(
                wo_all[:].rearrange("p (cp c) -> p cp c", c=C),
                wo.ap().rearrange("(cp p) c -> p cp c", p=128),
            )
            nc.sync.dma_start(bo_all[:], bo.ap().rearrange("(c p) -> p c", p=128))

            # flat F32 memsets only (strided / non-f32 memsets fail walrus
            # codegen); ones/zero fills below go through DVE broadcast copies
            ones_sb = pconst.tile([128, 24], F32, name=f"{P}ones", tag="ones")
            nc.gpsimd.memset(ones_sb[:], 1.0)
            zero_sb = pconst.tile([128, 24], F32, name=f"{P}zero", tag="zero")
            nc.gpsimd.memset(zero_sb[:], 0.0)

            # PE p-state warm-up: the clock ramps only while the PE stays
            # continuously busy, and the DMA-paced startup projections would
            # otherwise keep resetting it to the slow state. Dummy fp32
            # matmuls on a memset tile hold the ramp until the exp stream
            # starts (fp32 so no DMA dependency; flat F32 memset is the only
            # memset flavor walrus accepts).
            warm_sb = pconst.tile([128, 128], F32, name=f"{P}warm", tag="warm")
            nc.gpsimd.memset(warm_sb[:], 1.0)
            # tiny dummy exp so the 1.3us activation-table load happens at
            # t~0 instead of stalling the first real exp
            nc.scalar.activation(
                warm_sb[0:1, 127:128], warm_sb[0:1, 126:127],
                mybir.ActivationFunctionType.Exp,
            )
            warm_ps = psh.tile([128, 512], F32, name=f"{P}warmps", tag="sh")

            def warm(n):
                for _ in range(n):
                    nc.tensor.matmul(
                        warm_ps[:, 0:128], warm_sb[:], warm_sb[:],
                        start=True, stop=True,
                    )

            # ---- persistent activations ------------------------------
            kTp = [pqk.tile([128, S], F32R, name=f"{P}kTp{m}", tag=f"kTp{m}") for m in range(4)]
            qTp = [
                [pqk.tile([128, S], F32R, name=f"{P}qTp{f}_{m}", tag="qT", bufs=8) for m in range(4)]
                for f in range(FPC)
            ]
            # v: [t, 8*(40+1)] bf16 per t-tile; col h*41+40 is the ones col
            v_sb = [pvs.tile([128, H * VW], BF16, name=f"{P}v{tt}", tag=f"v{tt}") for tt in range(8)]
            for tt in range(8):
                nc.vector.tensor_copy(
                    v_sb[tt][:].rearrange("p (h w) -> p h w", w=VW)[:, :, D:VW],
                    ones_sb[:, 0:1].unsqueeze(1).broadcast_to([128, H, 1]),
                )
            # oT: [head-pair rows, s] per frame, padded 64/head
            oT = [
                [pot.tile([128, S], F32R, name=f"{P}oT{f}_{hp}", tag="oT", bufs=8) for hp in range(4)]
                for f in range(FPC)
            ]

            # ---- deferred projection groups --------------------------
            def proj_v(tt):
                """v[t, c-dense] for one t-tile; bf16 copy into 41-strided blocks."""
                def run():
                    ps = psh.tile([128, 512], F32, name=f"{P}ppv{tt}", tag="sh")
                    for ci in range(3):
                        nc.tensor.matmul(
                            ps[0:128, 0:C],
                            x0_sb[ci][:, tt * 128:(tt + 1) * 128],
                            wv_sb[ci][:],
                            start=(ci == 0),
                            stop=(ci == 2),
                        )
                    nc.vector.tensor_copy(
                        v_sb[tt][:].rearrange("p (h w) -> p h w", w=VW)[:, :, 0:D],
                        ps[0:128, 0:C].rearrange("p (h d) -> p h d", d=D),
                    )
                return run

            def proj_qk_quarter(dst_tiles, w_tiles, x_tiles, m, q):
                """One 256-col quarter of a 128-row [d-pad, s] projection."""
                def run():
                    ps = psh.tile([128, 512], F32, name=f"{P}pp{m}q{q}", tag="sh")
                    for ci in range(3):
                        nc.tensor.matmul(
                            ps[:, 0:256],
                            w_tiles[ci][:, m * 128:(m + 1) * 128],
                            x_tiles[ci][:, q * 256:(q + 1) * 256],
                            start=(ci == 0),
                            stop=(ci == 2),
                        )
                    nc.vector.tensor_copy(dst_tiles[m][:, q * 256:(q + 1) * 256], ps[:, 0:256])
                return run

            def k0_chunk(tc0, tc1):
                """k m0 t-cols [tc0*128, tc1*128) chasing the x0 chunk DMAs.
"""
                def run():
                    w = (tc1 - tc0) * 128
                    ps = psh.tile([128, 512], F32, name=f"{P}ppk0_{tc0}", tag="sh")
                    for ci in range(3):
                        nc.tensor.matmul(
                            ps[:, 0:w],
                            wk_sb[ci][:, 0:128],
                            x0_sb[ci][:, tc0 * 128:tc1 * 128],
                            start=(ci == 0),
                            stop=(ci == 2),
                        )
                    nc.vector.tensor_copy(kTp[0][:, tc0 * 128:tc1 * 128], ps[:, 0:w])
                return run

            extras = []
            extras.append(k0_chunk(2, 4))
            extras.append(proj_v(1))
            extras.append(proj_v(2))
            extras.append(k0_chunk(4, 6))
            extras.append(proj_v(3))
            extras.append(k0_chunk(6, 8))
            for tt in range(4, 8):
                extras.append(proj_v(tt))
            for m in (1, 2, 3):
                for q in range(4):
                    extras.append(proj_qk_quarter(kTp, wk_sb, x0_sb, m, q))
                    extras.append(proj_qk_quarter(qTp[0], wq_sb, xf_sb[0], m, q))

            def pop_extra():
                if extras:
                    extras.pop(0)()

            # ---- serial head: q m0 + k m0 t0:2 + v0, chasing DMAs, with
            # ---- warm-up matmuls sized to the DMA gaps so the PE p-state
            # ---- never resets (counts assume mid-clock 427ns per warm mm)
            warm(12)
            ps_q = psc.tile([128, S], F32, name=f"{P}ppq0", tag="sc")
            for ci in range(3):
                for sh in range(2):
                    nc.tensor.matmul(
                        ps_q[:, sh * 512:(sh + 1) * 512],
                        wq_sb[ci][:, 0:128],
                        xf_sb[0][ci][:, sh * 512:(sh + 1) * 512],
                        start=(ci == 0),
                        stop=(ci == 2),
                        skip_group_check=True,
                    )
                if ci < 2:
                    warm(5)
            nc.vector.tensor_copy(qTp[0][0][:, 0:512], ps_q[:, 0:512])
            # second half on the (idle) scalar engine, in parallel with DVE
            nc.scalar.activation(
                qTp[0][0][:, 512:1024], ps_q[:, 512:1024],
                mybir.ActivationFunctionType.Copy,
            )
            warm(1)
            k0_chunk(0, 2)()

            # ---- attention: software-pipelined (unit, tt) stream -----
            # PE is strictly in-order, so scores are emitted 2 steps ahead
            # of their exp, and all other PE work (PV epilogues, O-proj,
            # projection extras) is drip-fed between steps so the PE never
            # parks behind an Act dependency.
            steps = [(f, h, tt) for f in range(FPC) for h in range(H) for tt in range(8)]
            st_tiles: dict = {}
            pvz_tiles: dict = {}
            pt_tiles: dict = {}
            _tail_ps: dict = {}

            def get_pt(f, h):
                if (f, h) not in pt_tiles:
                    pt_tiles[(f, h)] = ppt.tile(
                        [128, 8 * S], BF16, name=f"{P}pt{f}{h}", tag="pt"
                    )
                return pt_tiles[(f, h)]

            def emit_scores(i):
                f, h, tt = steps[i]
                hp, hl = h // 2, (h % 2) * 64
                if tt in OFFLOAD_TT[f * H + h]:
                    # DVE fast-exp chunk: scores go through the psh pool so
                    # they never occupy a scores-pool slot - the slot WAR
                    # chain (the per-step pacing floor) skips this step and
                    # the Act exp stream compresses over the hole.
                    for sh in range(2):
                        stp = psh.tile([128, 512], F32, name=f"{P}sto{f}{h}{tt}{sh}", tag="sh")
                        nc.tensor.matmul(
                            stp[:],
                            kTp[hp][hl:hl + 64, tt * 128:(tt + 1) * 128],
                            qTp[f][hp][hl:hl + 64, sh * 512:(sh + 1) * 512],
                            start=True,
                            stop=True,
                            tile_position=(hl, 0),
                        )
                        nc.vector.tensor_scalar(
                            get_pt(f, h)[:, tt * S + sh * 512:tt * S + (sh + 1) * 512]
                            .bitcast(mybir.dt.int16),
                            stp[:],
                            SCH_A, SCH_B,
                            mybir.AluOpType.mult, mybir.AluOpType.add,
                        )
                    st_tiles[i] = None
                    return
                st = psc.tile([128, S], F32, name=f"{P}st{f}{h}{tt}", tag="sc")
                st_tiles[i] = st
                for sh in range(2):
                    nc.tensor.matmul(
                        st[:, sh * 512:(sh + 1) * 512],
                        kTp[hp][hl:hl + 64, tt * 128:(tt + 1) * 128],
                        qTp[f][hp][hl:hl + 64, sh * 512:(sh + 1) * 512],
                        start=True,
                        stop=True,
                        tile_position=(hl, 0),
                    )

            def transposes_half(f, h, o_u, half):
                hp, hl = h // 2, (h % 2) * 64
                if f == FPC - 1 and h == H - 1:
                    # last unit: psh is held by the pre-run O-proj groups;
                    # the freshly-drained scores slots are free instead
                    tp2 = psc.tile([128, S], F32R, name=f"{P}tp{f}{h}{half}", tag="sc")
                    tp = tp2[0:128, 0:512]
                else:
                    tp = psh.tile([128, 512], F32R, name=f"{P}tp{f}{h}{half}", tag="sh")
                for b in range(4):
                    sb = half * 4 + b
                    # single bank-group across the 4 blocks (see PV comment)
                    nc.tensor.matmul(
                        tp[0:64, b * 128:(b + 1) * 128],
                        o_u[:, sb * DP:sb * DP + DP],
                        ident_sb[:],
                        is_transpose=True,
                        start=(b == 0),
                        stop=(b == 3),
                    )
                nc.vector.tensor_copy(
                    oT[f][hp][hl:hl + 64, half * 512:(half + 1) * 512],
                    tp[0:64, :],
                )

            def oproj_mm(ps, f, m, sh, cp0, cp1, start, stop):
                cos, con = CO[m]
                for cp in range(cp0, cp1):
                    nc.tensor.matmul(
                        ps[:],
                        wo_sb[cp][:, cos:cos + con],
                        oT[f][cp][:, sh * 512:(sh + 1) * 512],
                        start=(start and cp == cp0),
                        stop=(stop and cp == cp1 - 1),
                    )

            def oproj_fin(ps, f, m, sh, use_act):
                cos, con = CO[m]
                y_sb = py.tile([con, 512], F32, name=f"{P}y{f}{m}{sh}", tag="y")
                if use_act:
                    nc.scalar.activation(
                        y_sb[:], ps[:],
                        mybir.ActivationFunctionType.Identity, bias=bo_sb[m][:],
                    )
                else:
                    nc.vector.tensor_scalar_add(y_sb[:], ps[:], bo_sb[m][:])
                nc.sync.dma_start(
                    yt.ap()[f, cos:cos + con, sh * 512:(sh + 1) * 512],
                    y_sb[:],
                )

            def oproj_group(f, m, sh):
                cos, con = CO[m]
                if f == FPC - 1:
                    ps2 = psc.tile([128, S], F32, name=f"{P}py{f}{m}{sh}", tag="sc")
                    ps = ps2[0:con, 0:512]
                else:
                    ps = psh.tile([con, 512], F32, name=f"{P}py{f}{m}{sh}", tag="sh")
                oproj_mm(ps, f, m, sh, 0, 4, True, True)
                oproj_fin(ps, f, m, sh, f == FPC - 1)

            def oproj_half_extras(f, m, sh):
                """Frame-0 steady-state: two ~430ns extras instead of one
                850ns pop so the PE never overruns the exp cadence."""
                state = {}

                def part1():
                    state["ps"] = psh.tile([CO[m][1], 512], F32, name=f"{P}py{f}{m}{sh}", tag="sh")
                    oproj_mm(state["ps"], f, m, sh, 0, 2, True, False)

                def part2():
                    oproj_mm(state["ps"], f, m, sh, 2, 4, False, True)
                    oproj_fin(state["ps"], f, m, sh, False)

                return part1, part2

            emit_scores(0)
            emit_scores(1)
            proj_v(0)()
            for i, (f, h, tt) in enumerate(steps):
                if tt == 0:
                    pvz_tiles[(f, h)] = ppv.tile(
                        [128, 8 * VW], F32, name=f"{P}pvz{f}{h}", tag="pvz"
                    )
                    get_pt(f, h)
                if tt != 7:
                    # popped BEFORE this step's scores emission so deferred
                    # projections always precede the scores that read them;
                    # double-pop early so v/k chunks outrun their consumers
                    pop_extra()
                    if i < 32 and i % 2 == 0:
                        pop_extra()
                st = st_tiles.pop(i)
                pt_u = pt_tiles[(f, h)]
                pvz = pvz_tiles[(f, h)]
                if st is not None:
                    nc.scalar.activation(
                        pt_u[:, tt * S:(tt + 1) * S], st[:],
                        mybir.ActivationFunctionType.Exp, scale=SCALE,
                    )
                if i + 2 < len(steps):
                    emit_scores(i + 2)
                # one accumulation group per PSUM BANK: the first matmul's
                # start=True zeroes the whole 2KB bank (all 8 sb regions),
                # everything after accumulates, the last one closes the group
                for sb in range(8):
                    nc.tensor.matmul(
                        pvz[:, sb * VW:(sb + 1) * VW],
                        pt_u[:, tt * S + sb * 128:tt * S + (sb + 1) * 128],
                        v_sb[tt][:, h * VW:(h + 1) * VW],
                        start=(tt == 0 and sb == 0),
                        stop=(tt == 7 and sb == 7),
                    )
                if tt == 7:
                    # normalize now (DVE only - does not block the PE queue)
                    pvz_v = pvz[:].rearrange("p (b w) -> p b w", w=VW)
                    rcz = prc.tile([128, 8], F32, name=f"{P}rcz{f}{h}", tag="rcz")
                    nc.vector.reciprocal(
                        rcz[:], pvz_v[:, :, D:VW].rearrange("p b w -> p (b w)")
                    )
                    o_u = po.tile([128, 8 * DP], F32R, name=f"{P}o{f}{h}", tag="o")
                    o_v = o_u[:].rearrange("p (b d) -> p b d", d=DP)
                    nc.vector.tensor_copy(
                        o_v[:, :, D:DP],
                        zero_sb[:].unsqueeze(1).broadcast_to([128, 8, DP - D]),
                    )
                    for bh in range(2):
                        nc.vector.tensor_mul(
                            o_v[:, bh * 4:(bh + 1) * 4, 0:D],
                            pvz_v[:, bh * 4:(bh + 1) * 4, 0:D],
                            rcz[:, bh * 4:(bh + 1) * 4].unsqueeze(2).broadcast_to([128, 4, D]),
                        )
                    del pvz_tiles[(f, h)], pt_tiles[(f, h)]
                    # PE-side epilogue is deferred a couple of steps
                    extras.append(lambda f=f, h=h, o_u=o_u: transposes_half(f, h, o_u, 0))
                    extras.append(lambda f=f, h=h, o_u=o_u: transposes_half(f, h, o_u, 1))
                    if f == 0 and h == H - 1:
                        for m in range(3):
                            for sh in range(2):
                                p1, p2 = oproj_half_extras(0, m, sh)
                                extras.append(p1)
                                extras.append(p2)
                    if f == 1 and h == H - 2:
                        # tail prep: start three frame-1 O-proj groups on the
                        # head-pairs that are already done (cp 0..2), using
                        # psum that is idle during the final unit
                        for gi, (m, sh) in enumerate([(0, 0), (0, 1), (1, 0)]):
                            def prerun(gi=gi, m=m, sh=sh):
                                con = CO[m][1]
                                if gi < 2:
                                    ps = psh.tile([con, 512], F32, name=f"{P}pyt{m}{sh}", tag="sh")
                                else:
                                    ps2 = ppv.tile([128, 512], F32, name=f"{P}pyt{m}{sh}", tag="pvz")
                                    ps = ps2[0:con, 0:512]
                                _tail_ps[(m, sh)] = ps
                                oproj_mm(ps, 1, m, sh, 0, 3, True, False)
                            extras.append(prerun)
                    if f == 0 and h == 0:
                        load_xf_frame(1)
                        for m in range(4):
                            for q in range(4):
                                extras.append(
                                    proj_qk_quarter(qTp[1], wq_sb, xf_sb[1], m, q)
                                )
            # ---- tail: finish the pre-run groups, then the rest ----------
            while extras:
                pop_extra()
            for (m, sh), ps in _tail_ps.items():
                oproj_mm(ps, 1, m, sh, 3, 4, False, True)
                oproj_fin(ps, 1, m, sh, True)
            for m, sh in [(1, 1), (2, 0), (2, 1)]:
                oproj_group(1, m, sh)

    nc.compile()
    return nc


def _get_nc(loop_n: int = 1):
    if loop_n not in _NC_CACHE:
        _NC_CACHE[loop_n] = _build(loop_n)
    return _NC_CACHE[loop_n]


def _pad_heads_cols(wT: np.ndarray) -> np.ndarray:
    """[C, C] (c_in, c_out) -> [C, CP] with each head's 40 cols at h*64."""
    out = np.zeros((C, CP), np.float32)
    out.reshape(C, H, DP)[:, :, :D] = wT.reshape(C, H, D)
    return out


def _prep_inputs(hidden_states, Wq, Wk, Wv, Wo, bo, video_length, k):
    hidden_states = np.asarray(hidden_states, dtype=np.float32)
    B = hidden_states.shape[0]
    assert hidden_states.shape == (B, S, C), hidden_states.shape
    assert B == NCORES * FPC, B
    kf = int(k)
    vl = int(video_length)
    b = B // vl
    assert b == 1, "kernel specialized for batch 1 (b*video_length == B)"

    xT = np.zeros((B, 384, S), np.float32)
    xT[:, :C, :] = hidden_states.transpose(0, 2, 1)
    wk_p = _pad_heads_cols(np.asarray(Wk, np.float32).T)
    wq_p = _pad_heads_cols(np.asarray(Wq, np.float32).T)
    wkq_p = np.zeros((384, 2 * CP), np.float32)
    wkq_p[:C] = np.concatenate([wk_p, wq_p], axis=1)
    wvd_p = np.zeros((384, C), np.float32)
    wvd_p[:C] = np.asarray(Wv, np.float32).T
    wo_p = np.zeros((CP, C), np.float32)
    wo_p.reshape(H, DP, C)[:, :D, :] = np.asarray(Wo, np.float32).T.reshape(H, D, C)
    bo_f = np.zeros(384, np.float32)
    bo_f[:C] = np.asarray(bo, np.float32)
    ident = np.eye(128, dtype=np.float32)

    xt0 = np.ascontiguousarray(xT[kf])
    in_maps = []
    for c in range(NCORES):
        in_maps.append(
            {
                "xt0": xt0,
                "xtf": np.ascontiguousarray(xT[c * FPC:(c + 1) * FPC]),
                "wkq": wkq_p,
                "wvd": wvd_p,
                "wo": wo_p,
                "bo": bo_f,
                "ident": ident,
            }
        )
    return in_maps


def _run(inputs: dict, loop_n: int = 1):
    global LAST_RESULTS
    nc = _get_nc(loop_n)
    in_maps = _prep_inputs(**inputs)
    last_exc = None
    for _attempt in range(3):
        try:
            res = bass_utils.run_bass_kernel_spmd(nc, in_maps, core_ids=list(range(NCORES)))
            break
        except Exception as e:  # transient NRT/axon device hiccups
            last_exc = e
            import time as _time
            _time.sleep(2.0)
    else:
        raise last_exc
    LAST_RESULTS = res
    B = NCORES * FPC
    y = np.empty((B, S, C), np.float32)
    for c in range(NCORES):
        y[c * FPC:(c + 1) * FPC] = res.results[c]["yt"].transpose(0, 2, 1)
    return y


def kernel(hidden_states, Wq, Wk, Wv, Wo, bo, video_length, k):
    return _run(
        dict(
            hidden_states=hidden_states,
            Wq=Wq,
            Wk=Wk,
            Wv=Wv,
            Wo=Wo,
            bo=bo,
            video_length=video_length,
            k=k,
        )
    )
